# revision 4
# baseline (speedup 1.0000x reference)
"""Trainium2 Bass kernel: 7x7 valid 2D cross-correlation of an 8192x8192
fp32 image plus scalar bias, row-sharded across 8 NeuronCores.

Formulation (per core): the y-direction 7-tap convolution for a fixed kernel
column dx is a banded matmul: out_dx[y, x] = sum_r A_dx[r, y] * X[r, x] with
A_dx[r, y] = K[r - y, dx].  The full conv accumulates the 7 dx terms in PSUM
with the moving operand (image columns) shifted by dx.  Matmuls run in bf16
(inputs bf16, fp32 PSUM accumulate); the banded weight blocks are padded to
128 columns so the compiler's fast-weight-load path engages.

Work distribution: 8186 output rows = 68 bands of <=122 rows.  Each core gets
8 full bands (rows 976*i .. 976*i+976) plus HALF of one of bands 64..67
(8 column tiles), i.e. 136 (band, col-tile) units/core instead of 9 full
bands = 144 — the PE-time quantum is a 512-column matmul pass, so the old
layout wasted 8 units/core on a mostly-empty 9th band.  The half-band is
processed FIRST: its input is only ~1 MB, so the PE starts as soon as the
DMA rings come up instead of waiting for a full 2.1 MB slab.  Output is
stored per 1024-column pair tile immediately after its PSUM drain, so the
kernel tail after the last matmul is one small store, not a 2 MB band store.
"""

import numpy as np
import ml_dtypes

import concourse.bass as bass
import concourse.mybir as mybir
from concourse.tile import TileContext
from concourse.bass_utils import run_bass_kernel_spmd

H = W = 8192
KH = KW = 7
OH = OW = H - KH + 1          # 8186
N_CORES = 8
BAND_IN = 128                 # input rows per matmul band (partition dim)
BAND_OUT = BAND_IN - KH + 1   # 122 output rows per band
APAD = 128                    # A block columns (padded from BAND_OUT for FWL)
COL_TILE = 512                # moving-operand free dim (one PSUM bank, fp32)
F32 = mybir.dt.float32
BF16 = mybir.dt.bfloat16

MAIN_BANDS = 8                # full bands per core
MAIN_OUT = MAIN_BANDS * BAND_OUT      # 976
MAIN_IN = MAIN_OUT + KH - 1           # 982
HALF_TILES = 8                # col tiles in the half band
HALF_OUT_COLS = HALF_TILES * COL_TILE # 4096
HALF_IN_COLS = HALF_OUT_COLS + 8      # 4104 (6-col halo, padded to 8)

# Results object of the most recent hardware run (for test harnesses).
LAST_RESULTS = None


def _split_multi_waits(nc):
    """Walrus in this toolchain accepts at most ONE sync-wait per
    instruction; Tile's scheduler may attach several.  Hoist the extras onto
    single-wait InstEventSemaphore instructions inserted just before, on the
    same engine stream (a sequence of waits = AND of the conditions)."""
    uid = 0
    for fn in nc.m.functions:
        for blk in fn.blocks:
            new_list = []
            for inst in blk.instructions:
                si = getattr(inst, "sync_info", None)
                if si is not None and si.on_wait and len(si.on_wait) > 1:
                    waits = list(si.on_wait)
                    for w in waits[:-1]:
                        ev = mybir.InstEventSemaphore(
                            name=f"wait_split_{uid}",
                            ins=[],
                            outs=[],
                            sync_info=mybir.SyncInfo(on_wait=[w], on_update=[]),
                        )
                        uid += 1
                        ev.engine = inst.engine
                        new_list.append(ev)
                    si.on_wait = [waits[-1]]
                new_list.append(inst)
            blk.instructions[:] = new_list


def _build_nc(bias_val):
    nc = bass.Bass()
    Xm = nc.declare_dram_parameter("Xm", [MAIN_IN, W], BF16, isOutput=False)
    Xh = nc.declare_dram_parameter("Xh", [BAND_IN, HALF_IN_COLS], BF16, isOutput=False)
    A = nc.declare_dram_parameter("A", [BAND_IN, KW * APAD], BF16, isOutput=False)
    Om = nc.declare_dram_parameter("Om", [MAIN_OUT, OW], BF16, isOutput=True)
    Oh = nc.declare_dram_parameter("Oh", [BAND_OUT, HALF_OUT_COLS], BF16, isOutput=True)

    with TileContext(nc) as tc:
        with (
            tc.tile_pool(name="const", bufs=1) as cpool,
            tc.tile_pool(name="hx", bufs=1) as hxpool,
            tc.tile_pool(name="x", bufs=4) as xpool,
            tc.tile_pool(name="o", bufs=6) as opool,
            tc.tile_pool(name="ps", bufs=8, space="PSUM") as pspool,
        ):
            # Startup: the DMA rings come up staggered; split the small gating
            # loads (A, half-band input) across all four issuing queues so the
            # first matmul is gated on ~1.3 MB of 4-way-parallel traffic, not a
            # serialized 2.1 MB slab.
            a_tile = cpool.tile([BAND_IN, KW * APAD], BF16)
            nc.sync.dma_start(out=a_tile[0:48, :], in_=A[0:48, :])
            nc.scalar.dma_start(out=a_tile[48:96, :], in_=A[48:96, :])
            nc.gpsimd.dma_start(out=a_tile[96:128, :], in_=A[96:128, :])

            # Half-band input in two column-halves: the first 4 col tiles only
            # gate on the first half.
            hx_a = hxpool.tile([BAND_IN, 2056], BF16, tag="hxa")
            hx_b = hxpool.tile([BAND_IN, HALF_IN_COLS - 2048], BF16, tag="hxb")
            nc.sync.dma_start(out=hx_a[0:64, :], in_=Xh[0:64, 0:2056])
            nc.scalar.dma_start(out=hx_a[64:128, :], in_=Xh[64:128, 0:2056])
            nc.gpsimd.dma_start(out=hx_b[0:64, :], in_=Xh[0:64, 2048:HALF_IN_COLS])
            nc.gpsimd.dma_start(out=hx_b[64:128, :], in_=Xh[64:128, 2048:HALF_IN_COLS])

            x_tiles = {}

            def issue_load(bi, split3=False):
                if bi >= MAIN_BANDS:
                    return
                r0 = bi * BAND_OUT
                xt = xpool.tile([BAND_IN, W], BF16, tag="x")
                if split3:
                    nc.gpsimd.dma_start(out=xt[0:64, :], in_=Xm[r0 : r0 + 64, :])
                    nc.sync.dma_start(out=xt[64:96, :], in_=Xm[r0 + 64 : r0 + 96, :])
                    nc.scalar.dma_start(out=xt[96:128, :], in_=Xm[r0 + 96 : r0 + 128, :])
                else:
                    nc.gpsimd.dma_start(out=xt[0:64, :], in_=Xm[r0 : r0 + 64, :])
                    nc.gpsimd.dma_start(out=xt[64:128, :], in_=Xm[r0 + 64 : r0 + 128, :])
                x_tiles[bi] = xt

            # Prefetch the first main bands behind the half-band gating loads.
            issue_load(0, split3=True)
            issue_load(1)
            issue_load(2)

            store_engs = (nc.sync, nc.scalar)
            store_idx = [0]

            def do_pair(x_tile, col_pairs, out_param, out_row0):
                """Process pairs of column tiles: 7 accumulating matmuls per
                tile, drain each into half of a pair o_tile, store the pair."""
                for x0a, x0b in col_pairs:
                    o_tile = opool.tile([BAND_OUT, 2 * COL_TILE], BF16, tag="o")
                    wa = min(COL_TILE, OW - x0a)
                    wb = min(COL_TILE, OW - x0b)
                    for x0, w, c0 in ((x0a, wa, 0), (x0b, wb, COL_TILE)):
                        ps = pspool.tile([APAD, COL_TILE], F32)
                        for dx in range(KW):
                            nc.tensor.matmul(
                                ps[:, :w],
                                lhsT=a_tile[:, dx * APAD : (dx + 1) * APAD],
                                rhs=x_tile[:, x0 + dx : x0 + dx + w],
                                start=(dx == 0),
                                stop=(dx == KW - 1),
                            )
                        nc.vector.tensor_scalar_add(
                            o_tile[:, c0 : c0 + w], ps[:BAND_OUT, :w], float(bias_val)
                        )
                    eng = store_engs[store_idx[0] % len(store_engs)]
                    store_idx[0] += 1
                    eng.dma_start(
                        out=out_param[out_row0 : out_row0 + BAND_OUT, x0a : x0b + wb],
                        in_=o_tile[:, : COL_TILE + wb],
                    )

            # --- half band first (small input => earliest possible PE start)
            do_pair(hx_a, [(0, 512), (1024, 1536)], Oh, 0)
            # tiles 4..7 read cols 2048..4103 of Xh => local cols in hx_b
            for x0a, x0b in ((2048, 2560), (3072, 3584)):
                # reindex into hx_b (starts at col 2048)
                o_tile = opool.tile([BAND_OUT, 2 * COL_TILE], BF16, tag="o")
                for x0, c0 in ((x0a, 0), (x0b, COL_TILE)):
                    ps = pspool.tile([APAD, COL_TILE], F32)
                    for dx in range(KW):
                        nc.tensor.matmul(
                            ps[:, :COL_TILE],
                            lhsT=a_tile[:, dx * APAD : (dx + 1) * APAD],
                            rhs=hx_b[:, x0 - 2048 + dx : x0 - 2048 + dx + COL_TILE],
                            start=(dx == 0),
                            stop=(dx == KW - 1),
                        )
                    nc.vector.tensor_scalar_add(
                        o_tile[:, c0 : c0 + COL_TILE],
                        ps[:BAND_OUT, :COL_TILE],
                        float(bias_val),
                    )
                eng = store_engs[store_idx[0] % len(store_engs)]
                store_idx[0] += 1
                eng.dma_start(
                    out=Oh[0:BAND_OUT, x0a : x0b + COL_TILE],
                    in_=o_tile[:, :],
                )

            # --- main bands
            main_pairs = [
                (2 * p * COL_TILE, (2 * p + 1) * COL_TILE) for p in range(8)
            ]
            for bi in range(MAIN_BANDS):
                issue_load(bi + 3)
                x_tile = x_tiles.pop(bi)
                do_pair(x_tile, main_pairs, Om, bi * BAND_OUT)

    _split_multi_waits(nc)
    return nc


def _make_A(K):
    A = np.zeros((BAND_IN, KW * APAD), np.float32)
    for dx in range(KW):
        for y in range(BAND_OUT):
            A[y : y + KH, dx * APAD + y] = K[:, dx]
    return A.astype(ml_dtypes.bfloat16)


def kernel(X, K, bias, _trace=False):
    global LAST_RESULTS
    X = np.asarray(X, dtype=np.float32)
    K = np.asarray(K, dtype=np.float32)
    bias_val = float(np.asarray(bias).reshape(-1)[0])

    A = _make_A(K)
    Xb = X.astype(ml_dtypes.bfloat16)

    in_maps = []
    for i in range(N_CORES):
        xm = Xb[MAIN_OUT * i : MAIN_OUT * i + MAIN_IN]  # contiguous view
        b = 64 + i // 2
        r0 = BAND_OUT * b
        rows = min(BAND_IN, H - r0)  # band 67 has only 18 real input rows
        xh = np.zeros((BAND_IN, HALF_IN_COLS), ml_dtypes.bfloat16)
        if i % 2 == 0:
            xh[:rows, :] = Xb[r0 : r0 + rows, 0:HALF_IN_COLS]
        else:
            xh[:rows, : W - 4096] = Xb[r0 : r0 + rows, 4096:W]
        in_maps.append({"Xm": xm, "Xh": xh, "A": A})

    nc = _build_nc(bias_val)
    res = run_bass_kernel_spmd(nc, in_maps, core_ids=list(range(N_CORES)), trace=_trace)
    LAST_RESULTS = res

    full = np.empty((OH, OW), np.float32)
    for i in range(N_CORES):
        full[MAIN_OUT * i : MAIN_OUT * (i + 1)] = res.results[i]["Om"].astype(
            np.float32
        )
        b = 64 + i // 2
        r0 = BAND_OUT * b
        nr = min(BAND_OUT, OH - r0)  # band 67: 12 valid rows
        oh = res.results[i]["Oh"].astype(np.float32)
        if i % 2 == 0:
            full[r0 : r0 + nr, 0:4096] = oh[:nr, :4096]
        else:
            full[r0 : r0 + nr, 4096:OW] = oh[:nr, : OW - 4096]
    return full


# revision 6
# speedup vs baseline: 1.8693x; 1.8693x over previous
"""Trainium2 Bass kernel: 7x7 valid 2D cross-correlation of an 8192x8192
fp32 image plus scalar bias, row-sharded across 8 NeuronCores.

Formulation (per core): the y-direction 7-tap convolution for a fixed kernel
column dx is a banded matmul: out_dx[y, x] = sum_r A_dx[r, y] * X[r, x] with
A_dx[r, y] = K[r - y, dx].  The full conv accumulates the 7 dx terms in PSUM
with the moving operand (image columns) shifted by dx.  Matmuls run in bf16
(inputs bf16, fp32 PSUM accumulate); the banded weight blocks are padded to
128 columns so the compiler's fast-weight-load path engages.

Work distribution: 8186 output rows = 68 bands of <=122 rows.  Each core gets
8 full bands (rows 976*i .. 976*i+976) plus HALF of one of bands 64..67
(8 column tiles), i.e. 136 (band, col-tile) units/core instead of 9 full
bands = 144 — the PE-time quantum is a 512-column matmul pass, so the old
layout wasted 8 units/core on a mostly-empty 9th band.  The half-band is
processed FIRST: its input is only ~1 MB, so the PE starts as soon as the
DMA rings come up instead of waiting for a full 2.1 MB slab.  Output is
stored per 1024-column pair tile immediately after its PSUM drain, so the
kernel tail after the last matmul is one small store, not a 2 MB band store.
"""

import numpy as np
import ml_dtypes

import concourse.bass as bass
import concourse.mybir as mybir
from concourse.tile import TileContext
from concourse.bass_utils import run_bass_kernel_spmd

H = W = 8192
KH = KW = 7
OH = OW = H - KH + 1          # 8186
N_CORES = 8
BAND_IN = 128                 # input rows per matmul band (partition dim)
BAND_OUT = BAND_IN - KH + 1   # 122 output rows per band
APAD = 128                    # A block columns (padded from BAND_OUT for FWL)
COL_TILE = 512                # moving-operand free dim (one PSUM bank, fp32)
F32 = mybir.dt.float32
BF16 = mybir.dt.bfloat16

MAIN_BANDS = 8                # full bands per core
MAIN_OUT = MAIN_BANDS * BAND_OUT      # 976
MAIN_IN = MAIN_OUT + KH - 1           # 982
HALF_TILES = 8                # col tiles in the half band
HALF_OUT_COLS = HALF_TILES * COL_TILE # 4096
HALF_IN_COLS = HALF_OUT_COLS + 8      # 4104 (6-col halo, padded to 8)

# Results object of the most recent hardware run (for test harnesses).
LAST_RESULTS = None


def _split_multi_waits(nc):
    """Walrus in this toolchain accepts at most ONE sync-wait per
    instruction; Tile's scheduler may attach several.  Hoist the extras onto
    single-wait InstEventSemaphore instructions inserted just before, on the
    same engine stream (a sequence of waits = AND of the conditions)."""
    uid = 0
    for fn in nc.m.functions:
        for blk in fn.blocks:
            new_list = []
            for inst in blk.instructions:
                si = getattr(inst, "sync_info", None)
                if si is not None and si.on_wait and len(si.on_wait) > 1:
                    waits = list(si.on_wait)
                    for w in waits[:-1]:
                        ev = mybir.InstEventSemaphore(
                            name=f"wait_split_{uid}",
                            ins=[],
                            outs=[],
                            sync_info=mybir.SyncInfo(on_wait=[w], on_update=[]),
                        )
                        uid += 1
                        ev.engine = inst.engine
                        new_list.append(ev)
                    si.on_wait = [waits[-1]]
                new_list.append(inst)
            blk.instructions[:] = new_list


def _build_nc(bias_val):
    nc = bass.Bass()
    Xm = nc.declare_dram_parameter("Xm", [MAIN_IN, W], BF16, isOutput=False)
    Xh = nc.declare_dram_parameter("Xh", [BAND_IN, HALF_IN_COLS], BF16, isOutput=False)
    A = nc.declare_dram_parameter("A", [BAND_IN, KW * APAD], BF16, isOutput=False)
    Om = nc.declare_dram_parameter("Om", [MAIN_OUT, OW], BF16, isOutput=True)
    Oh = nc.declare_dram_parameter("Oh", [BAND_OUT, HALF_OUT_COLS], BF16, isOutput=True)

    with TileContext(nc) as tc:
        with (
            tc.tile_pool(name="const", bufs=1) as cpool,
            tc.tile_pool(name="hx", bufs=1) as hxpool,
            tc.tile_pool(name="x", bufs=4) as xpool,
            tc.tile_pool(name="o", bufs=3) as opool,
            tc.tile_pool(name="ps", bufs=8, space="PSUM") as pspool,
        ):
            # Startup: the DMA rings come up staggered; split the small gating
            # loads (A, half-band input) across all four issuing queues so the
            # first matmul is gated on ~1.3 MB of 4-way-parallel traffic, not a
            # serialized 2.1 MB slab.
            a_tile = cpool.tile([BAND_IN, KW * APAD], BF16)
            nc.sync.dma_start(out=a_tile[0:48, :], in_=A[0:48, :])
            nc.scalar.dma_start(out=a_tile[48:96, :], in_=A[48:96, :])
            nc.gpsimd.dma_start(out=a_tile[96:128, :], in_=A[96:128, :])

            # Half-band input in two column-halves: the first 4 col tiles only
            # gate on the first half.
            hx_a = hxpool.tile([BAND_IN, 2056], BF16, tag="hxa")
            hx_b = hxpool.tile([BAND_IN, HALF_IN_COLS - 2048], BF16, tag="hxb")
            nc.sync.dma_start(out=hx_a[0:64, :], in_=Xh[0:64, 0:2056])
            nc.scalar.dma_start(out=hx_a[64:128, :], in_=Xh[64:128, 0:2056])
            nc.gpsimd.dma_start(out=hx_b[0:64, :], in_=Xh[0:64, 2048:HALF_IN_COLS])
            nc.gpsimd.dma_start(out=hx_b[64:128, :], in_=Xh[64:128, 2048:HALF_IN_COLS])

            x_tiles = {}

            def issue_load(bi, split3=False):
                if bi >= MAIN_BANDS:
                    return
                r0 = bi * BAND_OUT
                xt = xpool.tile([BAND_IN, W], BF16, tag="x")
                if split3:
                    nc.gpsimd.dma_start(out=xt[0:64, :], in_=Xm[r0 : r0 + 64, :])
                    nc.sync.dma_start(out=xt[64:96, :], in_=Xm[r0 + 64 : r0 + 96, :])
                    nc.scalar.dma_start(out=xt[96:128, :], in_=Xm[r0 + 96 : r0 + 128, :])
                else:
                    nc.gpsimd.dma_start(out=xt[0:64, :], in_=Xm[r0 : r0 + 64, :])
                    nc.gpsimd.dma_start(out=xt[64:128, :], in_=Xm[r0 + 64 : r0 + 128, :])
                x_tiles[bi] = xt

            # Prefetch the first main bands behind the half-band gating loads.
            issue_load(0, split3=True)
            issue_load(1)
            issue_load(2)

            def conv_tile(x_tile, x0, w, o_tile, c0):
                """7 accumulating matmuls into a PSUM bank, drain to o_tile."""
                ps = pspool.tile([APAD, COL_TILE], F32)
                for dx in range(KW):
                    nc.tensor.matmul(
                        ps[:, :w],
                        lhsT=a_tile[:, dx * APAD : (dx + 1) * APAD],
                        rhs=x_tile[:, x0 + dx : x0 + dx + w],
                        start=(dx == 0),
                        stop=(dx == KW - 1),
                    )
                nc.vector.tensor_scalar_add(
                    o_tile[:, c0 : c0 + w], ps[:BAND_OUT, :w], float(bias_val)
                )

            # --- half band first (small input => earliest possible PE start).
            # Stores go on sync/scalar (HWDGE) with full 4096-col rows =>
            # 8 KB packets; gpsimd keeps streaming main-band loads meanwhile.
            o_half = opool.tile([BAND_OUT, HALF_OUT_COLS], BF16, tag="oh")
            for j in range(4):
                conv_tile(hx_a, j * COL_TILE, COL_TILE, o_half, j * COL_TILE)
            for j in range(4, HALF_TILES):
                conv_tile(
                    hx_b, j * COL_TILE - 2048, COL_TILE, o_half, j * COL_TILE
                )
            nc.sync.dma_start(out=Oh[0:61, :], in_=o_half[0:61, :])
            nc.scalar.dma_start(out=Oh[61:BAND_OUT, :], in_=o_half[61:BAND_OUT, :])

            # --- main bands.  Per-band wide o_tile, stored as full-width row
            # chunks (16 KB packets) spread across the three DMA queues.
            for bi in range(MAIN_BANDS):
                issue_load(bi + 3)
                x_tile = x_tiles.pop(bi)
                o_tile = opool.tile([BAND_OUT, OW], BF16, tag="om")
                for j in range(16):
                    x0 = j * COL_TILE
                    w = min(COL_TILE, OW - x0)
                    conv_tile(x_tile, x0, w, o_tile, x0)
                s = bi * BAND_OUT
                nchunks = 8
                bounds = [(BAND_OUT * k) // nchunks for k in range(nchunks + 1)]
                for k in range(nchunks):
                    p0, p1 = bounds[k], bounds[k + 1]
                    eng = (nc.gpsimd, nc.sync, nc.gpsimd, nc.scalar)[k % 4]
                    eng.dma_start(out=Om[s + p0 : s + p1, :], in_=o_tile[p0:p1, :])

    _split_multi_waits(nc)
    return nc


def _make_A(K):
    A = np.zeros((BAND_IN, KW * APAD), np.float32)
    for dx in range(KW):
        for y in range(BAND_OUT):
            A[y : y + KH, dx * APAD + y] = K[:, dx]
    return A.astype(ml_dtypes.bfloat16)


def kernel(X, K, bias, _trace=False):
    global LAST_RESULTS
    X = np.asarray(X, dtype=np.float32)
    K = np.asarray(K, dtype=np.float32)
    bias_val = float(np.asarray(bias).reshape(-1)[0])

    A = _make_A(K)
    Xb = X.astype(ml_dtypes.bfloat16)

    in_maps = []
    for i in range(N_CORES):
        xm = Xb[MAIN_OUT * i : MAIN_OUT * i + MAIN_IN]  # contiguous view
        b = 64 + i // 2
        r0 = BAND_OUT * b
        rows = min(BAND_IN, H - r0)  # band 67 has only 18 real input rows
        xh = np.zeros((BAND_IN, HALF_IN_COLS), ml_dtypes.bfloat16)
        if i % 2 == 0:
            xh[:rows, :] = Xb[r0 : r0 + rows, 0:HALF_IN_COLS]
        else:
            xh[:rows, : W - 4096] = Xb[r0 : r0 + rows, 4096:W]
        in_maps.append({"Xm": xm, "Xh": xh, "A": A})

    nc = _build_nc(bias_val)
    res = run_bass_kernel_spmd(nc, in_maps, core_ids=list(range(N_CORES)), trace=_trace)
    LAST_RESULTS = res

    full = np.empty((OH, OW), np.float32)
    for i in range(N_CORES):
        full[MAIN_OUT * i : MAIN_OUT * (i + 1)] = res.results[i]["Om"].astype(
            np.float32
        )
        b = 64 + i // 2
        r0 = BAND_OUT * b
        nr = min(BAND_OUT, OH - r0)  # band 67: 12 valid rows
        oh = res.results[i]["Oh"].astype(np.float32)
        if i % 2 == 0:
            full[r0 : r0 + nr, 0:4096] = oh[:nr, :4096]
        else:
            full[r0 : r0 + nr, 4096:OW] = oh[:nr, : OW - 4096]
    return full
